# revision 24
# baseline (speedup 1.0000x reference)
"""Trainium2 Bass kernel for nn_AttentionHead (B=16, S=2048, D=1024, H=128).

Single attention head with key-mask + deterministic dropout (jax key 42).
Data-parallel over batch: 2 batch elements per core on 8 cores.

Per-core layout strategy (all matmuls contract over the partition dim):
  - x is pre-transposed on host to xT [D, S] so QKV projections run directly.
  - Q^T, K^T, V^T produced as [H=128 part, S free] (c-outer accumulation so
    the PE starts as soon as the first xT chunk lands); V^T is transposed
    on the PE to V [k part, H free] for the AV matmul.
  - scores^T [k part, q free] = K_blk @ Q^T  (one 128-contraction matmul).
  - attn mask folds into exp's per-partition bias (-1e9 on masked k).
  - E = exp(scores^T + bias); denominator = ones-matmul over E (PSUM f32).
  - dropout = precomputed {0, 1/(1-p)} bf16 mask, 4 batched DMAs per q-block.
  - out^T [H, q] = sum_k V_blk^T @ (E*mask); transpose + per-q 1/denom scale.

qb/kb/vb are zeros per the problem spec (asserted on host) and folded out.
"""

import math

import numpy as np
import ml_dtypes

B, S, D, H = 16, 2048, 1024, 128
N_CORES = 8
BPC = B // N_CORES  # batches per core
QB = 512            # query block (free dim per matmul)
NQB = S // QB
NKB = S // 128      # key blocks of 128
NC = D // 128       # contraction chunks
DROPOUT_P = 0.3
QSCALE = 1.0 / math.sqrt(H)

_BF16 = ml_dtypes.bfloat16


def _build_bass():
    import concourse.mybir as mybir
    import concourse.tile as tile
    from concourse import bacc
    from concourse.masks import make_identity

    f32 = mybir.dt.float32
    bf16 = mybir.dt.bfloat16
    AF = mybir.ActivationFunctionType

    nc = bacc.Bacc(None, target_bir_lowering=False, debug=False)

    xT = nc.dram_tensor("xT", [BPC, D, S], bf16, kind="ExternalInput")
    wT = nc.dram_tensor("wT", [3, D, H], bf16, kind="ExternalInput")
    biask = nc.dram_tensor("biask", [BPC, 128, NKB], f32, kind="ExternalInput")
    maskR = nc.dram_tensor(
        "maskR", [BPC, NQB, 128, NKB, QB], bf16, kind="ExternalInput"
    )
    out = nc.dram_tensor("out", [BPC, S, H], f32, kind="ExternalOutput")

    with tile.TileContext(nc) as tc:
        with (
            tc.tile_pool(name="singles", bufs=1) as singles,
            tc.tile_pool(name="work", bufs=6) as work,
            tc.tile_pool(name="mwork", bufs=3) as mwork,
            tc.tile_pool(name="ep", bufs=2) as ep,
            tc.tile_pool(name="psum_mm", bufs=2, space="PSUM") as psum_mm,
            tc.tile_pool(name="psum_acc", bufs=4, space="PSUM") as psum_acc,
            tc.tile_pool(name="psum_tp", bufs=2, space="PSUM") as psum_tp,
        ):
            # ---- constants / weights ----
            ident = singles.tile([128, 128], f32)
            make_identity(nc, ident)
            identb = singles.tile([128, 128], bf16)
            make_identity(nc, identb)
            ones = singles.tile([128, 128], bf16)
            nc.vector.memset(ones, 1.0)

            w_sb = singles.tile([128, 3, NC, H], bf16)
            wTr = wT.rearrange("w (c p) h -> p w c h", p=128)
            nc.sync.dma_start(w_sb[:, 0], wTr[:, 0])  # qw first: needed first

            # ---- x^T resident in SBUF: [128, BPC, NC, S] bf16 ----
            xts = singles.tile([128, BPC, NC, S], bf16)
            for b in range(BPC):
                for c in range(NC):
                    if b == 0 and c == 0:
                        # split the very first chunk so the first projection
                        # matmul can start ~3us earlier
                        for mi in range(4):
                            nc.sync.dma_start(
                                xts[:, 0, 0, mi * QB : (mi + 1) * QB],
                                xT[0, 0:128, mi * QB : (mi + 1) * QB],
                            )
                    else:
                        nc.sync.dma_start(
                            xts[:, b, c, :], xT[b, c * 128 : (c + 1) * 128, :]
                        )
                if b == 0:
                    nc.sync.dma_start(w_sb[:, 1], wTr[:, 1])
                    nc.sync.dma_start(w_sb[:, 2], wTr[:, 2])
            qw_sb, kw_sb, vw_sb = w_sb[:, 0], w_sb[:, 1], w_sb[:, 2]

            bias_sb = singles.tile([128, BPC, NKB], f32)
            nc.sync.dma_start(bias_sb, biask.rearrange("b p t -> p b t"))

            QT = singles.tile([128, BPC, S], bf16)  # [H, q]
            KT = singles.tile([128, BPC, S], bf16)  # [H, k]
            Vs = singles.tile([128, BPC, S], bf16)  # [k(part), 16 x H]

            # ---- per batch: projections then attention, so batch 1's x
            # DMA and projection hide under batch 0's attention ----
            for b in range(BPC):
                # Q and K interleaved per x-chunk: 8 matmuls per chunk
                # outpace the chunk DMA period, so the PE stays dense
                # through the DMA-paced start. K borrows the idle mm/tp
                # PSUM slots since Q owns the 4 acc slots.
                pssQ = [
                    psum_acc.tile([128, QB], f32, tag="acc", name=f"accq{i}")
                    for i in range(NQB)
                ]
                pssK = [
                    psum_mm.tile([128, QB], f32, tag="mm", name=f"kmm{i}")
                    for i in range(2)
                ] + [
                    psum_tp.tile([128, QB], f32, tag="tp", name=f"ktp{i}")
                    for i in range(2)
                ]
                for c in range(NC):
                    for cb in range(NQB):
                        nc.tensor.matmul(
                            pssQ[cb],
                            lhsT=qw_sb[:, c, :],
                            rhs=xts[:, b, c, cb * QB : (cb + 1) * QB],
                            start=(c == 0),
                            stop=(c == NC - 1),
                        )
                    for cb in range(NQB):
                        nc.tensor.matmul(
                            pssK[cb],
                            lhsT=kw_sb[:, c, :],
                            rhs=xts[:, b, c, cb * QB : (cb + 1) * QB],
                            start=(c == 0),
                            stop=(c == NC - 1),
                        )
                for cb in range(NQB):
                    nc.scalar.activation(
                        QT[:, b, cb * QB : (cb + 1) * QB],
                        pssQ[cb], AF.Copy, scale=QSCALE,
                    )
                for cb in range(NQB):
                    nc.scalar.activation(
                        KT[:, b, cb * QB : (cb + 1) * QB],
                        pssK[cb], AF.Copy, scale=1.0,
                    )
                # V^T then PE-transpose into [k, H] layout
                vts = work.tile([128, S], bf16, tag="VT")
                pss = [
                    psum_acc.tile([128, QB], f32, tag="acc", name=f"acc{i}")
                    for i in range(NQB)
                ]
                for c in range(NC):
                    for cb in range(NQB):
                        nc.tensor.matmul(
                            pss[cb],
                            lhsT=vw_sb[:, c, :],
                            rhs=xts[:, b, c, cb * QB : (cb + 1) * QB],
                            start=(c == 0),
                            stop=(c == NC - 1),
                        )
                for cb in range(NQB):
                    nc.scalar.activation(
                        vts[:, cb * QB : (cb + 1) * QB], pss[cb], AF.Copy
                    )
                for t in range(S // 128):
                    ptv = psum_tp.tile([128, 128], bf16, tag="tp")
                    nc.tensor.transpose(
                        ptv, vts[:, t * 128 : (t + 1) * 128], identb
                    )
                    nc.vector.tensor_copy(Vs[:, b, t * 128 : (t + 1) * 128], ptv)

                # ---- attention for this batch ----
                def emit_epilogue(b, qb, dn_sb, oc_sb):
                    # PE transposes + normalize + store; deferred into the
                    # next q-block's stream so the PE never idles on the
                    # PSUM-evacuation copies. All 4 transposes of each kind
                    # land in ONE psum tile so the PE never waits on DVE.
                    rT = ep.tile([128, QB // 128], f32, tag="rT", name="rT")
                    dt_ps = psum_tp.tile([128, QB], bf16, tag="tp",
                                         name="dtps")
                    ot_ps = psum_tp.tile([128, QB], bf16, tag="tp",
                                         name="otps")
                    nt = QB // 128
                    for t in range(nt):
                        nc.tensor.transpose(
                            dt_ps[:, t * 128 : (t + 1) * 128],
                            dn_sb[:, t * 128 : (t + 1) * 128], identb
                        )
                    for t in range(nt):
                        nc.tensor.transpose(
                            ot_ps[:, t * 128 : (t + 1) * 128],
                            oc_sb[:, t * 128 : (t + 1) * 128], identb
                        )
                    dn_cols = dt_ps.rearrange("p (t h) -> p t h", h=128)[:, :, 0]
                    nc.vector.reciprocal(rT, dn_cols)
                    for t in range(nt):
                        of = ep.tile([128, 128], f32, tag="of", name="of")
                        nc.vector.tensor_scalar_mul(
                            of, ot_ps[:, t * 128 : (t + 1) * 128],
                            rT[:, t : t + 1],
                        )
                        r0 = qb * QB + t * 128
                        nc.sync.dma_start(out[b, r0 : r0 + 128, :], of)

                pend = None
                for qb in range(NQB):
                    Mq = mwork.tile([128, NKB, QB], bf16, tag="M")
                    for mi in range(4):
                        nc.sync.dma_start(
                            Mq[:, mi * (NKB // 4) : (mi + 1) * (NKB // 4), :],
                            maskR[b, qb, :, mi * (NKB // 4) : (mi + 1) * (NKB // 4), :],
                        )
                    dn_ps = psum_acc.tile([128, QB], f32, tag="acc")
                    o_ps = psum_acc.tile([128, QB], f32, tag="acc")
                    qs = QT[:, b, qb * QB : (qb + 1) * QB]
                    # software pipeline: scores at lag 0, denominator at
                    # lag 2 (hides exp latency), AV at lag 3 (hides the
                    # DVE dropout multiply)
                    Es, E2s = {}, {}
                    for kb in range(NKB + 3):
                        if kb < NKB:
                            sc = psum_mm.tile([128, QB], f32, tag="mm")
                            nc.tensor.matmul(
                                sc,
                                lhsT=KT[:, b, kb * 128 : (kb + 1) * 128],
                                rhs=qs,
                                start=True,
                                stop=True,
                            )
                            E = work.tile([128, QB], bf16, tag="E")
                            nc.scalar.activation(
                                E, sc, AF.Exp,
                                bias=bias_sb[:, b, kb : kb + 1], scale=1.0,
                            )
                            Es[kb] = E
                            E2 = work.tile([128, QB], bf16, tag="E2")
                            nc.vector.tensor_mul(E2, E, Mq[:, kb, :])
                            E2s[kb] = E2
                        if kb == 3 and pend is not None:
                            pend()
                            pend = None
                        j = kb - 2
                        if 0 <= j < NKB:
                            nc.tensor.matmul(
                                dn_ps, lhsT=ones, rhs=Es.pop(j),
                                start=(j == 0), stop=(j == NKB - 1),
                            )
                        j = kb - 3
                        if j >= 0:
                            nc.tensor.matmul(
                                o_ps,
                                lhsT=Vs[:, b, j * 128 : (j + 1) * 128],
                                rhs=E2s.pop(j),
                                start=(j == 0),
                                stop=(j == NKB - 1),
                            )
                    # PSUM evacuation now; PE-side epilogue deferred
                    dn_sb = ep.tile([128, QB], bf16, tag="dn_sb")
                    nc.vector.tensor_copy(dn_sb, dn_ps)
                    oc_sb = ep.tile([128, QB], bf16, tag="oc_sb")
                    nc.scalar.copy(oc_sb, o_ps)
                    pend = (lambda b=b, qb=qb, d=dn_sb, o=oc_sb:
                            emit_epilogue(b, qb, d, o))
                if pend is not None:
                    pend()
                    pend = None
    nc.compile()
    return nc


def _host_prep(x, attention_mask, qw, qb, kw, kb, vw, vb):
    """Build per-core input maps (list of dicts keyed by dram tensor name)."""
    import jax

    x = np.asarray(x, dtype=np.float32)
    attention_mask = np.asarray(attention_mask)
    qw = np.asarray(qw, dtype=np.float32)
    kw = np.asarray(kw, dtype=np.float32)
    vw = np.asarray(vw, dtype=np.float32)
    for name, bias in (("qb", qb), ("kb", kb), ("vb", vb)):
        assert not np.any(np.asarray(bias)), f"{name} expected to be zero"

    # x^T per batch: [B, D, S] bf16
    xT = np.ascontiguousarray(x.transpose(0, 2, 1)).astype(_BF16)

    # dropout keep mask, bit-exact with the reference (fixed key 42), on CPU
    cpu = jax.devices("cpu")[0]
    with jax.default_device(cpu):
        keep = jax.random.bernoulli(
            jax.random.key(42), 1.0 - DROPOUT_P, (B, S, S)
        )
        keep = np.asarray(keep)  # bool [B, q, k]
    keepT = keep.transpose(0, 2, 1)  # [B, k, q]
    scale = _BF16(1.0 / (1.0 - DROPOUT_P))
    maskT = np.where(keepT, scale, _BF16(0.0))  # bf16 [B, k, q]
    # regroup per (qblock): [B, NQB, 128(k mod), NKB, QB] with contiguous
    # 16KB-per-partition runs for single-DMA loading
    maskR = np.ascontiguousarray(
        maskT.reshape(B, NKB, 128, NQB, QB).transpose(0, 3, 2, 1, 4)
    )

    # additive attention bias per k: 0 keep, -1e9 masked; layout [B, 128, NKB]
    bias = np.where(attention_mask == 0, np.float32(-1e9), np.float32(0.0))
    bias_r = np.ascontiguousarray(
        bias.reshape(B, NKB, 128).transpose(0, 2, 1)
    ).astype(np.float32)

    wT = np.ascontiguousarray(
        np.stack([qw.T, kw.T, vw.T])
    ).astype(_BF16)

    in_maps = []
    for c in range(N_CORES):
        lo, hi = c * BPC, (c + 1) * BPC
        in_maps.append(
            dict(
                xT=np.ascontiguousarray(xT[lo:hi]),
                wT=wT,
                biask=np.ascontiguousarray(bias_r[lo:hi]),
                maskR=np.ascontiguousarray(maskR[lo:hi]),
            )
        )
    return in_maps


def _ensure_ntff_hook():
    """bass_utils imports antenv.axon_hooks whenever tracing is requested
    (including via the BASS_TRACE env var). That module is absent in this
    image; provide it, backed by the ctypes NTFF profiler from the axon
    boot shim when available."""
    import sys
    import types

    try:
        import antenv.axon_hooks  # noqa: F401
        return
    except ImportError:
        pass
    try:
        import antenv
    except ImportError:
        return
    hook = None
    try:
        from trn_agent_boot.trn_boot import _ntff_profile_via_ctypes

        hook = _ntff_profile_via_ctypes("/opt/axon/libaxon_pjrt.so")
    except Exception:
        hook = None
    mod = types.ModuleType("antenv.axon_hooks")
    mod.get_axon_ntff_profile_hook = lambda: hook
    mod.set_axon_ntff_profile_hook = lambda h: None
    sys.modules["antenv.axon_hooks"] = mod
    antenv.axon_hooks = mod


def run(inputs, trace=False, trace_cores=None):
    """Build, run on 8 cores, return (full_output, BassKernelResults)."""
    from concourse.bass_utils import run_bass_kernel_spmd

    _ensure_ntff_hook()

    in_maps = _host_prep(**inputs)
    nc = _build_bass()
    res = run_bass_kernel_spmd(
        nc,
        in_maps,
        core_ids=list(range(N_CORES)),
        trace=trace,
        trace_cores=trace_cores,
    )
    outs = [r["out"] for r in res.results]
    full = np.concatenate(outs, axis=0).astype(np.float32)
    return full, res


def kernel(**inputs) -> np.ndarray:
    full, _ = run(inputs, trace=False)
    return full


# revision 25
# speedup vs baseline: 1.0353x; 1.0353x over previous
"""Trainium2 Bass kernel for nn_AttentionHead (B=16, S=2048, D=1024, H=128).

Single attention head with key-mask + deterministic dropout (jax key 42).
Data-parallel over batch: 2 batch elements per core on 8 cores.

Per-core layout strategy (all matmuls contract over the partition dim):
  - x is pre-transposed on host to xT [D, S] so QKV projections run directly.
  - Q^T, K^T, V^T produced as [H=128 part, S free] (c-outer accumulation so
    the PE starts as soon as the first xT chunk lands); V^T is transposed
    on the PE to V [k part, H free] for the AV matmul.
  - scores^T [k part, q free] = K_blk @ Q^T  (one 128-contraction matmul).
  - attn mask folds into exp's per-partition bias (-1e9 on masked k).
  - E = exp(scores^T + bias); denominator = ones-matmul over E (PSUM f32).
  - dropout = precomputed {0, 1/(1-p)} bf16 mask, 4 batched DMAs per q-block.
  - out^T [H, q] = sum_k V_blk^T @ (E*mask); transpose + per-q 1/denom scale.

qb/kb/vb are zeros per the problem spec (asserted on host) and folded out.
"""

import math

import numpy as np
import ml_dtypes

B, S, D, H = 16, 2048, 1024, 128
N_CORES = 8
BPC = B // N_CORES  # batches per core
QB = 512            # query block (free dim per matmul)
NQB = S // QB
NKB = S // 128      # key blocks of 128
NC = D // 128       # contraction chunks
DROPOUT_P = 0.3
QSCALE = 1.0 / math.sqrt(H)

_BF16 = ml_dtypes.bfloat16


def _build_bass():
    import concourse.mybir as mybir
    import concourse.tile as tile
    from concourse import bacc
    from concourse.masks import make_identity

    f32 = mybir.dt.float32
    bf16 = mybir.dt.bfloat16
    AF = mybir.ActivationFunctionType

    nc = bacc.Bacc(None, target_bir_lowering=False, debug=False)

    xT = nc.dram_tensor("xT", [BPC, D, S], bf16, kind="ExternalInput")
    wT = nc.dram_tensor("wT", [3, D, H], bf16, kind="ExternalInput")
    biask = nc.dram_tensor("biask", [BPC, 128, NKB], f32, kind="ExternalInput")
    maskR = nc.dram_tensor(
        "maskR", [BPC, NQB, 128, NKB, QB], bf16, kind="ExternalInput"
    )
    out = nc.dram_tensor("out", [BPC, S, H], f32, kind="ExternalOutput")

    with tile.TileContext(nc) as tc:
        with (
            tc.tile_pool(name="singles", bufs=1) as singles,
            tc.tile_pool(name="work", bufs=6) as work,
            tc.tile_pool(name="mwork", bufs=3) as mwork,
            tc.tile_pool(name="ep", bufs=2) as ep,
            tc.tile_pool(name="psum_mm", bufs=2, space="PSUM") as psum_mm,
            tc.tile_pool(name="psum_acc", bufs=4, space="PSUM") as psum_acc,
            tc.tile_pool(name="psum_tp", bufs=2, space="PSUM") as psum_tp,
        ):
            # ---- constants / weights ----
            ident = singles.tile([128, 128], f32)
            make_identity(nc, ident)
            identb = singles.tile([128, 128], bf16)
            make_identity(nc, identb)
            ones = singles.tile([128, 128], bf16)
            nc.vector.memset(ones, 1.0)

            w_sb = singles.tile([128, 3, NC, H], bf16)
            wTr = wT.rearrange("w (c p) h -> p w c h", p=128)
            nc.sync.dma_start(w_sb[:, 0], wTr[:, 0])  # qw first: needed first

            # ---- x^T resident in SBUF: [128, BPC, NC, S] bf16 ----
            xts = singles.tile([128, BPC, NC, S], bf16)
            for b in range(BPC):
                for c in range(NC):
                    if b == 0 and c == 0:
                        # split the very first chunk so the first projection
                        # matmul can start ~3us earlier
                        for mi in range(4):
                            nc.sync.dma_start(
                                xts[:, 0, 0, mi * QB : (mi + 1) * QB],
                                xT[0, 0:128, mi * QB : (mi + 1) * QB],
                            )
                    else:
                        nc.sync.dma_start(
                            xts[:, b, c, :], xT[b, c * 128 : (c + 1) * 128, :]
                        )
                if b == 0:
                    nc.sync.dma_start(w_sb[:, 1], wTr[:, 1])
                    nc.sync.dma_start(w_sb[:, 2], wTr[:, 2])
            qw_sb, kw_sb, vw_sb = w_sb[:, 0], w_sb[:, 1], w_sb[:, 2]

            bias_sb = singles.tile([128, BPC, NKB], f32)
            nc.sync.dma_start(bias_sb, biask.rearrange("b p t -> p b t"))

            QT = singles.tile([128, BPC, S], bf16)  # [H, q]
            KT = singles.tile([128, BPC, S], bf16)  # [H, k]
            Vs = singles.tile([128, BPC, S], bf16)  # [k(part), 16 x H]

            # ---- per batch: projections then attention, so batch 1's x
            # DMA and projection hide under batch 0's attention ----
            for b in range(BPC):
                for w_sb, o_sb, scale in (
                    (qw_sb, QT, QSCALE),
                    (kw_sb, KT, 1.0),
                ):
                    pss = [
                        psum_acc.tile([128, QB], f32, tag="acc", name=f"acc{i}")
                        for i in range(NQB)
                    ]
                    for c in range(NC):
                        for cb in range(NQB):
                            nc.tensor.matmul(
                                pss[cb],
                                lhsT=w_sb[:, c, :],
                                rhs=xts[:, b, c, cb * QB : (cb + 1) * QB],
                                start=(c == 0),
                                stop=(c == NC - 1),
                            )
                    for cb in range(NQB):
                        nc.scalar.activation(
                            o_sb[:, b, cb * QB : (cb + 1) * QB],
                            pss[cb],
                            AF.Copy,
                            scale=scale,
                        )
                # V^T then PE-transpose into [k, H] layout
                vts = work.tile([128, S], bf16, tag="VT")
                pss = [
                    psum_acc.tile([128, QB], f32, tag="acc", name=f"acc{i}")
                    for i in range(NQB)
                ]
                for c in range(NC):
                    for cb in range(NQB):
                        nc.tensor.matmul(
                            pss[cb],
                            lhsT=vw_sb[:, c, :],
                            rhs=xts[:, b, c, cb * QB : (cb + 1) * QB],
                            start=(c == 0),
                            stop=(c == NC - 1),
                        )
                for cb in range(NQB):
                    nc.scalar.activation(
                        vts[:, cb * QB : (cb + 1) * QB], pss[cb], AF.Copy
                    )
                for t in range(S // 128):
                    ptv = psum_tp.tile([128, 128], bf16, tag="tp")
                    nc.tensor.transpose(
                        ptv, vts[:, t * 128 : (t + 1) * 128], identb
                    )
                    nc.vector.tensor_copy(Vs[:, b, t * 128 : (t + 1) * 128], ptv)

                # ---- attention for this batch ----
                def emit_epilogue(b, qb, dn_sb, oc_sb):
                    # PE transposes + normalize + store; deferred into the
                    # next q-block's stream so the PE never idles on the
                    # PSUM-evacuation copies. All 4 transposes of each kind
                    # land in ONE psum tile so the PE never waits on DVE.
                    rT = ep.tile([128, QB // 128], f32, tag="rT", name="rT")
                    dt_ps = psum_tp.tile([128, QB], bf16, tag="tp",
                                         name="dtps")
                    ot_ps = psum_tp.tile([128, QB], bf16, tag="tp",
                                         name="otps")
                    nt = QB // 128
                    for t in range(nt):
                        nc.tensor.transpose(
                            dt_ps[:, t * 128 : (t + 1) * 128],
                            dn_sb[:, t * 128 : (t + 1) * 128], identb
                        )
                    for t in range(nt):
                        nc.tensor.transpose(
                            ot_ps[:, t * 128 : (t + 1) * 128],
                            oc_sb[:, t * 128 : (t + 1) * 128], identb
                        )
                    dn_cols = dt_ps.rearrange("p (t h) -> p t h", h=128)[:, :, 0]
                    nc.vector.reciprocal(rT, dn_cols)
                    for t in range(nt):
                        of = ep.tile([128, 128], f32, tag="of", name="of")
                        nc.vector.tensor_scalar_mul(
                            of, ot_ps[:, t * 128 : (t + 1) * 128],
                            rT[:, t : t + 1],
                        )
                        r0 = qb * QB + t * 128
                        nc.sync.dma_start(out[b, r0 : r0 + 128, :], of)

                pend = None
                for qb in range(NQB):
                    Mq = mwork.tile([128, NKB, QB], bf16, tag="M")
                    for mi in range(4):
                        nc.sync.dma_start(
                            Mq[:, mi * (NKB // 4) : (mi + 1) * (NKB // 4), :],
                            maskR[b, qb, :, mi * (NKB // 4) : (mi + 1) * (NKB // 4), :],
                        )
                    dn_ps = psum_acc.tile([128, QB], f32, tag="acc")
                    o_ps = psum_acc.tile([128, QB], f32, tag="acc")
                    qs = QT[:, b, qb * QB : (qb + 1) * QB]
                    # software pipeline: scores at lag 0, denominator at
                    # lag 2 (hides exp latency), AV at lag 3 (hides the
                    # DVE dropout multiply)
                    Es, E2s = {}, {}
                    for kb in range(NKB + 3):
                        if kb < NKB:
                            sc = psum_mm.tile([128, QB], f32, tag="mm")
                            nc.tensor.matmul(
                                sc,
                                lhsT=KT[:, b, kb * 128 : (kb + 1) * 128],
                                rhs=qs,
                                start=True,
                                stop=True,
                            )
                            E = work.tile([128, QB], bf16, tag="E")
                            nc.scalar.activation(
                                E, sc, AF.Exp,
                                bias=bias_sb[:, b, kb : kb + 1], scale=1.0,
                            )
                            Es[kb] = E
                            E2 = work.tile([128, QB], bf16, tag="E2")
                            nc.vector.tensor_mul(E2, E, Mq[:, kb, :])
                            E2s[kb] = E2
                        if kb == 3 and pend is not None:
                            pend()
                            pend = None
                        j = kb - 2
                        if 0 <= j < NKB:
                            nc.tensor.matmul(
                                dn_ps, lhsT=ones, rhs=Es.pop(j),
                                start=(j == 0), stop=(j == NKB - 1),
                            )
                        j = kb - 3
                        if j >= 0:
                            nc.tensor.matmul(
                                o_ps,
                                lhsT=Vs[:, b, j * 128 : (j + 1) * 128],
                                rhs=E2s.pop(j),
                                start=(j == 0),
                                stop=(j == NKB - 1),
                            )
                    # PSUM evacuation now; PE-side epilogue deferred
                    dn_sb = ep.tile([128, QB], bf16, tag="dn_sb")
                    nc.vector.tensor_copy(dn_sb, dn_ps)
                    oc_sb = ep.tile([128, QB], bf16, tag="oc_sb")
                    nc.scalar.copy(oc_sb, o_ps)
                    pend = (lambda b=b, qb=qb, d=dn_sb, o=oc_sb:
                            emit_epilogue(b, qb, d, o))
                if pend is not None:
                    pend()
                    pend = None
    nc.compile()
    return nc


def _host_prep(x, attention_mask, qw, qb, kw, kb, vw, vb):
    """Build per-core input maps (list of dicts keyed by dram tensor name)."""
    import jax

    x = np.asarray(x, dtype=np.float32)
    attention_mask = np.asarray(attention_mask)
    qw = np.asarray(qw, dtype=np.float32)
    kw = np.asarray(kw, dtype=np.float32)
    vw = np.asarray(vw, dtype=np.float32)
    for name, bias in (("qb", qb), ("kb", kb), ("vb", vb)):
        assert not np.any(np.asarray(bias)), f"{name} expected to be zero"

    # x^T per batch: [B, D, S] bf16
    xT = np.ascontiguousarray(x.transpose(0, 2, 1)).astype(_BF16)

    # dropout keep mask, bit-exact with the reference (fixed key 42), on CPU
    cpu = jax.devices("cpu")[0]
    with jax.default_device(cpu):
        keep = jax.random.bernoulli(
            jax.random.key(42), 1.0 - DROPOUT_P, (B, S, S)
        )
        keep = np.asarray(keep)  # bool [B, q, k]
    keepT = keep.transpose(0, 2, 1)  # [B, k, q]
    scale = _BF16(1.0 / (1.0 - DROPOUT_P))
    maskT = np.where(keepT, scale, _BF16(0.0))  # bf16 [B, k, q]
    # regroup per (qblock): [B, NQB, 128(k mod), NKB, QB] with contiguous
    # 16KB-per-partition runs for single-DMA loading
    maskR = np.ascontiguousarray(
        maskT.reshape(B, NKB, 128, NQB, QB).transpose(0, 3, 2, 1, 4)
    )

    # additive attention bias per k: 0 keep, -1e9 masked; layout [B, 128, NKB]
    bias = np.where(attention_mask == 0, np.float32(-1e9), np.float32(0.0))
    bias_r = np.ascontiguousarray(
        bias.reshape(B, NKB, 128).transpose(0, 2, 1)
    ).astype(np.float32)

    wT = np.ascontiguousarray(
        np.stack([qw.T, kw.T, vw.T])
    ).astype(_BF16)

    in_maps = []
    for c in range(N_CORES):
        lo, hi = c * BPC, (c + 1) * BPC
        in_maps.append(
            dict(
                xT=np.ascontiguousarray(xT[lo:hi]),
                wT=wT,
                biask=np.ascontiguousarray(bias_r[lo:hi]),
                maskR=np.ascontiguousarray(maskR[lo:hi]),
            )
        )
    return in_maps


def _ensure_ntff_hook():
    """bass_utils imports antenv.axon_hooks whenever tracing is requested
    (including via the BASS_TRACE env var). That module is absent in this
    image; provide it, backed by the ctypes NTFF profiler from the axon
    boot shim when available."""
    import sys
    import types

    try:
        import antenv.axon_hooks  # noqa: F401
        return
    except ImportError:
        pass
    try:
        import antenv
    except ImportError:
        return
    hook = None
    try:
        from trn_agent_boot.trn_boot import _ntff_profile_via_ctypes

        hook = _ntff_profile_via_ctypes("/opt/axon/libaxon_pjrt.so")
    except Exception:
        hook = None
    mod = types.ModuleType("antenv.axon_hooks")
    mod.get_axon_ntff_profile_hook = lambda: hook
    mod.set_axon_ntff_profile_hook = lambda h: None
    sys.modules["antenv.axon_hooks"] = mod
    antenv.axon_hooks = mod


def run(inputs, trace=False, trace_cores=None):
    """Build, run on 8 cores, return (full_output, BassKernelResults)."""
    from concourse.bass_utils import run_bass_kernel_spmd

    _ensure_ntff_hook()

    in_maps = _host_prep(**inputs)
    nc = _build_bass()
    res = run_bass_kernel_spmd(
        nc,
        in_maps,
        core_ids=list(range(N_CORES)),
        trace=trace,
        trace_cores=trace_cores,
    )
    outs = [r["out"] for r in res.results]
    full = np.concatenate(outs, axis=0).astype(np.float32)
    return full, res


def kernel(**inputs) -> np.ndarray:
    full, _ = run(inputs, trace=False)
    return full
